# revision 10
# baseline (speedup 1.0000x reference)
"""Trainium2 Bass kernel for DenseEquivariantMatrix.

Math:  out[b, fo, g] = sum_{fi,h} x[b, fi, h] * kernel[fo, fi, pt[h, g]] + bias[fo]

A B x K x N matmul (K = fi*h = 8192, N = fo*g = 8192) whose weight matrix is a
gather of 32x32 blocks from the kernel table.  Sharding: tensor-parallel over
the output n_symm dim (32 g's per core, 8 cores).

Per-core dataflow (bf16 operands, fp32 PSUM accumulation):
  - host converts x (pre-transposed) and the kernel table to bf16; the
    product-table gather runs on-device as 64 indirect DMAs (one whole 2KB
    bf16 kernel-table block per partition), hc-major.  Gather issue is
    ~1.4us each on gpsimd, so the matmul schedule chases the gather front:
    slab 0 starts on an 8-g quarter panel, pass 1 (h-half 1) runs as two
    half-width panel passes over all 16 b-slabs, and pass 2 (h-half 2)
    runs full-width once all of G is resident.
  - matmul rhs is a strided 3D AP into G at fixed (hc, fi): [h x g x fo]
    columns; lhsT is an X^T chunk [h x b].  bf16 weights get FWL, so the
    per-matmul LDWEIGHTS (~97ns) hides under the 512-col stream (~216ns).
  - K accumulated in PSUM over 32 fi-chunks per h-half; h-half 2 adds bias
    via a K=1 ones^T @ bias_row matmul and accumulates into DRAM with a
    SWDGE accum_op=add DMA.
"""

import os
import numpy as np

B = 2048
F_IN = 32
F_OUT = 32
H = 256  # n_symm (contraction copy)
G = 256  # n_symm (output copy)
N_CORES = 8
G_CORE = G // N_CORES  # 32
K = F_IN * H  # 8192
N_COLS = G_CORE * F_OUT  # 1024 per core, cols ordered (g_local, fo)
BLK = F_IN * F_OUT  # 1024 elements per kernel-table block

TRACE = bool(int(os.environ.get("KERNEL_TRACE", "0")))
LAST_RESULTS = None

_PROGRAM = None


def _build_program():
    import concourse.bacc as bacc
    import concourse.bass as bass
    import concourse.mybir as mybir
    import concourse.tile as tile

    f32 = mybir.dt.float32
    bf16 = mybir.dt.bfloat16
    i32 = mybir.dt.int32

    nc = bacc.Bacc(
        "TRN2", target_bir_lowering=False, debug=False, num_devices=N_CORES
    )

    # host-tiled X^T: xt[hc, m, p, fi, j] = x[m*128+j, fi, hc*128+p]
    # -> per (hc, m) slab, each partition p reads 8KB contiguous (bf16)
    xt = nc.dram_tensor(
        "xt", (2, B // 128, 128, F_IN, 128), bf16, kind="ExternalInput"
    ).ap()
    kt = nc.dram_tensor("kt", (H, BLK), bf16, kind="ExternalInput").ap()
    ptg = nc.dram_tensor("ptg", (H, G_CORE), i32, kind="ExternalInput").ap()
    biasrow = nc.dram_tensor("biasrow", (1, N_COLS), f32, kind="ExternalInput").ap()
    out = nc.dram_tensor("out", (B, N_COLS), f32, kind="ExternalOutput").ap()

    M_BLK = B // 128  # 16

    with tile.TileContext(nc) as tc:
        with (
            tc.tile_pool(name="const", bufs=1) as const_pool,
            tc.tile_pool(name="g", bufs=1) as g_pool,
            tc.tile_pool(name="x", bufs=4) as x_pool,
            tc.tile_pool(name="oh", bufs=3) as oh_pool,
            tc.tile_pool(name="of", bufs=2) as of_pool,
            tc.tile_pool(name="psh", bufs=3, space="PSUM") as psh_pool,
            tc.tile_pool(name="psf", bufs=2, space="PSUM") as psf_pool,
        ):
            # pts[p, hc*32+g] = pt[hc*128+p, g]
            pts = const_pool.tile([128, 2 * G_CORE], i32, tag="pts")
            nc.sync.dma_start(
                pts[:].rearrange("p (hc g) -> p hc g", hc=2),
                ptg.rearrange("(hc p) g -> p hc g", p=128),
            )
            # bias broadcast to all partitions, added during PSUM
            # evacuation on the vector engine (off the tensor engine)
            bias_sb = const_pool.tile([128, N_COLS], f32, tag="bias")
            nc.scalar.dma_start(bias_sb[:], biasrow.to_broadcast((128, N_COLS)))

            # whole G resident in SBUF: 128KB/partition in bf16
            Gt = g_pool.tile([128, 2 * G_CORE * BLK], bf16, tag="G")
            for hc in range(2):
                for g in range(G_CORE):
                    gg = hc * G_CORE + g
                    nc.gpsimd.indirect_dma_start(
                        out=Gt[:, gg * BLK : (gg + 1) * BLK],
                        out_offset=None,
                        in_=kt[:],
                        in_offset=bass.IndirectOffsetOnAxis(
                            ap=pts[:, gg : gg + 1], axis=0
                        ),
                    )
            G4 = Gt[:].rearrange(
                "p (hc g fi fo) -> p hc g fi fo", hc=2, g=G_CORE, fi=F_IN
            )

            def load_x(hc, m):
                xsl = x_pool.tile([128, F_IN * 128], bf16, tag="x")
                nc.sync.dma_start(
                    xsl[:], xt[hc, m].rearrange("p fi j -> p (fi j)")
                )
                return xsl

            def mm_panel(ps_ap, xsl, hc, g0, g1, start, stop):
                # accumulate x^T @ G[hc, g0:g1] over all fi into ps_ap
                for fi in range(F_IN):
                    nc.tensor.matmul(
                        ps_ap,
                        lhsT=xsl[:, fi * 128 : (fi + 1) * 128],
                        rhs=G4[:, hc, g0:g1, fi, :],
                        start=start and fi == 0,
                        stop=stop and fi == F_IN - 1,
                    )

            def evac_half(m, c0, cols, ps_ap):
                ot = oh_pool.tile([128, 512], f32, tag="oh")
                nc.vector.tensor_copy(ot[:, 0:cols], ps_ap)
                nc.sync.dma_start(
                    out[m * 128 : (m + 1) * 128, c0 : c0 + cols], ot[:, 0:cols]
                )

            # ---- pass 1 (hc=0): chase the gather front ----
            # gather blocks land ~1.5us apart (issue-serialized SWDGE, one
            # DMA engine per block); quarter panels cost the same per
            # column as halves (LDWEIGHTS hides in the reorder window), so
            # run panel A entirely as 8-g quarters in arrival order, with
            # 4-g eighths to start slab 0 as early as possible
            for m, g0, g1 in [(0, 0, 4), (0, 4, 8)] + [
                (m, 0, 8) for m in range(1, M_BLK - 1)
            ] + [(m, 8, 16) for m in range(M_BLK - 1)]:
                cols = (g1 - g0) * F_OUT
                xsl = load_x(0, m)
                ps = psh_pool.tile([128, 512], f32, tag="psh")
                mm_panel(ps[:, 0:cols], xsl, 0, g0, g1, True, True)
                evac_half(m, g0 * F_OUT, cols, ps[:, 0:cols])
            # panel B (g 16:32) halves for slabs 0..14
            for m in range(M_BLK - 1):
                xsl = load_x(0, m)
                ps = psh_pool.tile([128, 512], f32, tag="psh")
                mm_panel(ps[:], xsl, 0, 16, 32, True, True)
                evac_half(m, 512, 512, ps[:])

            # ---- pass 2 (hc=1): full width + bias, accumulate into DRAM ----
            for m in range(M_BLK - 1):
                xsl = load_x(1, m)
                ps = psf_pool.tile([128, N_COLS], f32, tag="psf")
                for fi in range(F_IN):
                    lhsT = xsl[:, fi * 128 : (fi + 1) * 128]
                    nc.tensor.matmul(
                        ps[:, 0:512], lhsT=lhsT,
                        rhs=G4[:, 1, 0:16, fi, :],
                        start=(fi == 0), stop=(fi == F_IN - 1),
                    )
                    nc.tensor.matmul(
                        ps[:, 512:1024], lhsT=lhsT,
                        rhs=G4[:, 1, 16:32, fi, :],
                        start=(fi == 0), stop=(fi == F_IN - 1),
                    )
                ot = of_pool.tile([128, N_COLS], f32, tag="of")
                nc.vector.tensor_add(ot[:], ps[:], bias_sb[:])
                nc.gpsimd.dma_start(
                    out[m * 128 : (m + 1) * 128, :], ot[:],
                    accum_op=mybir.AluOpType.add,
                )

            # ---- last slab full-K (both h-halves) with a plain final
            # write, column-split so colA's evacuation overlaps colB's
            # matmuls and the gpsimd accum queue drains early ----
            m = M_BLK - 1
            xsl0 = load_x(0, m)
            xsl1 = load_x(1, m)
            for half in range(2):
                ps = psh_pool.tile([128, 512], f32, tag="psh")
                for hc, xs in ((0, xsl0), (1, xsl1)):
                    for fi in range(F_IN):
                        nc.tensor.matmul(
                            ps[:],
                            lhsT=xs[:, fi * 128 : (fi + 1) * 128],
                            rhs=G4[:, hc, half * 16 : (half + 1) * 16, fi, :],
                            start=(hc == 0 and fi == 0),
                            stop=(hc == 1 and fi == F_IN - 1),
                        )
                ot = oh_pool.tile([128, 512], f32, tag="oh")
                nc.vector.tensor_add(
                    ot[:, 0:512], ps[:],
                    bias_sb[:, half * 512 : (half + 1) * 512],
                )
                nc.sync.dma_start(
                    out[m * 128 : (m + 1) * 128, half * 512 : (half + 1) * 512],
                    ot[:, 0:512],
                )

    nc.compile()
    return nc


def _get_program():
    global _PROGRAM
    if _PROGRAM is None:
        _PROGRAM = _build_program()
    return _PROGRAM


def kernel(x, kernel, bias, product_table):
    global LAST_RESULTS
    import ml_dtypes
    from concourse import bass_utils

    bf = ml_dtypes.bfloat16
    x = np.asarray(x, dtype=np.float32)
    kernel = np.asarray(kernel, dtype=np.float32)
    bias = np.asarray(bias, dtype=np.float32)
    product_table = np.asarray(product_table, dtype=np.int32)

    nc = _get_program()

    # host-tiled X^T: xt[hc, m, p, fi, j] = x[m*128+j, fi, hc*128+p]
    xt = np.ascontiguousarray(
        x.astype(bf).reshape(B // 128, 128, F_IN, 2, 128).transpose(3, 0, 4, 2, 1)
    )
    # kernel table KT[k][fi][fo]
    kt = np.ascontiguousarray(
        kernel.astype(bf).transpose(2, 1, 0)
    ).reshape(H, BLK)
    bias_row = np.ascontiguousarray(np.tile(bias, G_CORE)[None, :])

    in_maps = []
    for c in range(N_CORES):
        in_maps.append(
            {
                "xt": xt,
                "kt": kt,
                "ptg": np.ascontiguousarray(
                    product_table[:, c * G_CORE : (c + 1) * G_CORE]
                ),
                "biasrow": bias_row,
            }
        )

    res = bass_utils.run_bass_kernel_spmd(
        nc,
        in_maps,
        core_ids=list(range(N_CORES)),
        trace=TRACE,
        trace_cores=[0] if TRACE else None,
        tmpdir=os.environ.get("KERNEL_TMPDIR") or None,
    )
    LAST_RESULTS = res

    # per-core cols are (g_local, fo); assemble to (B, F_OUT, G)
    parts = [
        res.results[c]["out"].reshape(B, G_CORE, F_OUT).transpose(0, 2, 1)
        for c in range(N_CORES)
    ]
    return np.ascontiguousarray(np.concatenate(parts, axis=2), dtype=np.float32)


# revision 11
# speedup vs baseline: 1.0284x; 1.0284x over previous
"""Trainium2 Bass kernel for DenseEquivariantMatrix.

Math:  out[b, fo, g] = sum_{fi,h} x[b, fi, h] * kernel[fo, fi, pt[h, g]] + bias[fo]

A B x K x N matmul (K = fi*h = 8192, N = fo*g = 8192) whose weight matrix is a
gather of 32x32 blocks from the kernel table.  Sharding: tensor-parallel over
the output n_symm dim (32 g's per core, 8 cores).

Per-core dataflow (bf16 operands, fp32 PSUM accumulation):
  - host converts x (pre-transposed) and the kernel table to bf16; the
    product-table gather runs on-device as 64 indirect DMAs (one whole 2KB
    bf16 kernel-table block per partition), hc-major.  Gather issue is
    ~1.4us each on gpsimd, so the matmul schedule chases the gather front:
    slab 0 starts on an 8-g quarter panel, pass 1 (h-half 1) runs as two
    half-width panel passes over all 16 b-slabs, and pass 2 (h-half 2)
    runs full-width once all of G is resident.
  - matmul rhs is a strided 3D AP into G at fixed (hc, fi): [h x g x fo]
    columns; lhsT is an X^T chunk [h x b].  bf16 weights get FWL, so the
    per-matmul LDWEIGHTS (~97ns) hides under the 512-col stream (~216ns).
  - K accumulated in PSUM over 32 fi-chunks per h-half; h-half 2 adds bias
    via a K=1 ones^T @ bias_row matmul and accumulates into DRAM with a
    SWDGE accum_op=add DMA.
"""

import os
import numpy as np

B = 2048
F_IN = 32
F_OUT = 32
H = 256  # n_symm (contraction copy)
G = 256  # n_symm (output copy)
N_CORES = 8
G_CORE = G // N_CORES  # 32
K = F_IN * H  # 8192
N_COLS = G_CORE * F_OUT  # 1024 per core, cols ordered (g_local, fo)
BLK = F_IN * F_OUT  # 1024 elements per kernel-table block

TRACE = bool(int(os.environ.get("KERNEL_TRACE", "0")))
LAST_RESULTS = None

_PROGRAM = None


def _build_program():
    import concourse.bacc as bacc
    import concourse.bass as bass
    import concourse.mybir as mybir
    import concourse.tile as tile

    f32 = mybir.dt.float32
    bf16 = mybir.dt.bfloat16
    i32 = mybir.dt.int32

    nc = bacc.Bacc(
        "TRN2", target_bir_lowering=False, debug=False, num_devices=N_CORES
    )

    # host-tiled X^T: xt[hc, m, p, fi, j] = x[m*128+j, fi, hc*128+p]
    # -> per (hc, m) slab, each partition p reads 8KB contiguous (bf16)
    xt = nc.dram_tensor(
        "xt", (2, B // 128, 128, F_IN, 128), bf16, kind="ExternalInput"
    ).ap()
    kt = nc.dram_tensor("kt", (H, BLK), bf16, kind="ExternalInput").ap()
    ptg = nc.dram_tensor("ptg", (H, G_CORE), i32, kind="ExternalInput").ap()
    biasrow = nc.dram_tensor("biasrow", (1, N_COLS), f32, kind="ExternalInput").ap()
    out = nc.dram_tensor("out", (B, N_COLS), f32, kind="ExternalOutput").ap()

    M_BLK = B // 128  # 16

    with tile.TileContext(nc) as tc:
        with (
            tc.tile_pool(name="const", bufs=1) as const_pool,
            tc.tile_pool(name="g", bufs=1) as g_pool,
            tc.tile_pool(name="x", bufs=4) as x_pool,
            tc.tile_pool(name="oh", bufs=3) as oh_pool,
            tc.tile_pool(name="of", bufs=2) as of_pool,
            tc.tile_pool(name="psh", bufs=3, space="PSUM") as psh_pool,
            tc.tile_pool(name="psf", bufs=2, space="PSUM") as psf_pool,
        ):
            # pts[p, hc*32+g] = pt[hc*128+p, g]
            pts = const_pool.tile([128, 2 * G_CORE], i32, tag="pts")
            nc.sync.dma_start(
                pts[:].rearrange("p (hc g) -> p hc g", hc=2),
                ptg.rearrange("(hc p) g -> p hc g", p=128),
            )
            # bias broadcast to all partitions, added during PSUM
            # evacuation on the vector engine (off the tensor engine)
            bias_sb = const_pool.tile([128, N_COLS], f32, tag="bias")
            nc.scalar.dma_start(bias_sb[:], biasrow.to_broadcast((128, N_COLS)))

            # whole G resident in SBUF: 128KB/partition in bf16
            Gt = g_pool.tile([128, 2 * G_CORE * BLK], bf16, tag="G")
            for hc in range(2):
                for g in range(G_CORE):
                    gg = hc * G_CORE + g
                    nc.gpsimd.indirect_dma_start(
                        out=Gt[:, gg * BLK : (gg + 1) * BLK],
                        out_offset=None,
                        in_=kt[:],
                        in_offset=bass.IndirectOffsetOnAxis(
                            ap=pts[:, gg : gg + 1], axis=0
                        ),
                    )
            G4 = Gt[:].rearrange(
                "p (hc g fi fo) -> p hc g fi fo", hc=2, g=G_CORE, fi=F_IN
            )

            def load_x(hc, m):
                xsl = x_pool.tile([128, F_IN * 128], bf16, tag="x")
                nc.sync.dma_start(
                    xsl[:], xt[hc, m].rearrange("p fi j -> p (fi j)")
                )
                return xsl

            def mm_panel(ps_ap, xsl, hc, g0, g1, start, stop):
                # accumulate x^T @ G[hc, g0:g1] over all fi into ps_ap
                for fi in range(F_IN):
                    nc.tensor.matmul(
                        ps_ap,
                        lhsT=xsl[:, fi * 128 : (fi + 1) * 128],
                        rhs=G4[:, hc, g0:g1, fi, :],
                        start=start and fi == 0,
                        stop=stop and fi == F_IN - 1,
                    )

            def evac_half(m, c0, cols, ps_ap):
                # write on the (otherwise idle) scalar queue so pending
                # writes never head-of-line-block later x loads on sync
                ot = oh_pool.tile([128, 512], f32, tag="oh")
                nc.vector.tensor_copy(ot[:, 0:cols], ps_ap)
                nc.scalar.dma_start(
                    out[m * 128 : (m + 1) * 128, c0 : c0 + cols], ot[:, 0:cols]
                )

            # ---- pass 1 (hc=0): chase the gather front ----
            # gather blocks land ~1.5us apart (issue-serialized SWDGE, one
            # DMA engine per block); quarter panels cost the same per
            # column as halves (LDWEIGHTS hides in the reorder window), so
            # run panel A entirely as 8-g quarters in arrival order, with
            # 4-g eighths to start slab 0 as early as possible
            for m, g0, g1 in [(0, 0, 4), (0, 4, 8)] + [
                (m, 0, 8) for m in range(1, M_BLK - 1)
            ] + [(m, 8, 16) for m in range(M_BLK - 1)]:
                cols = (g1 - g0) * F_OUT
                xsl = load_x(0, m)
                ps = psh_pool.tile([128, 512], f32, tag="psh")
                mm_panel(ps[:, 0:cols], xsl, 0, g0, g1, True, True)
                evac_half(m, g0 * F_OUT, cols, ps[:, 0:cols])
            # panel B (g 16:32) halves for slabs 0..14
            for m in range(M_BLK - 1):
                xsl = load_x(0, m)
                ps = psh_pool.tile([128, 512], f32, tag="psh")
                mm_panel(ps[:], xsl, 0, 16, 32, True, True)
                evac_half(m, 512, 512, ps[:])

            # ---- pass 2 (hc=1): full width + bias, accumulate into DRAM ----
            for m in range(M_BLK - 1):
                xsl = load_x(1, m)
                ps = psf_pool.tile([128, N_COLS], f32, tag="psf")
                for fi in range(F_IN):
                    lhsT = xsl[:, fi * 128 : (fi + 1) * 128]
                    nc.tensor.matmul(
                        ps[:, 0:512], lhsT=lhsT,
                        rhs=G4[:, 1, 0:16, fi, :],
                        start=(fi == 0), stop=(fi == F_IN - 1),
                    )
                    nc.tensor.matmul(
                        ps[:, 512:1024], lhsT=lhsT,
                        rhs=G4[:, 1, 16:32, fi, :],
                        start=(fi == 0), stop=(fi == F_IN - 1),
                    )
                ot = of_pool.tile([128, N_COLS], f32, tag="of")
                nc.vector.tensor_add(ot[:], ps[:], bias_sb[:])
                nc.gpsimd.dma_start(
                    out[m * 128 : (m + 1) * 128, :], ot[:],
                    accum_op=mybir.AluOpType.add,
                )

            # ---- last slab full-K (both h-halves) with a plain final
            # write, column-split so colA's evacuation overlaps colB's
            # matmuls and the gpsimd accum queue drains early ----
            m = M_BLK - 1
            xsl0 = load_x(0, m)
            xsl1 = load_x(1, m)
            for half in range(2):
                ps = psh_pool.tile([128, 512], f32, tag="psh")
                for hc, xs in ((0, xsl0), (1, xsl1)):
                    for fi in range(F_IN):
                        nc.tensor.matmul(
                            ps[:],
                            lhsT=xs[:, fi * 128 : (fi + 1) * 128],
                            rhs=G4[:, hc, half * 16 : (half + 1) * 16, fi, :],
                            start=(hc == 0 and fi == 0),
                            stop=(hc == 1 and fi == F_IN - 1),
                        )
                ot = oh_pool.tile([128, 512], f32, tag="oh")
                nc.vector.tensor_add(
                    ot[:, 0:512], ps[:],
                    bias_sb[:, half * 512 : (half + 1) * 512],
                )
                nc.sync.dma_start(
                    out[m * 128 : (m + 1) * 128, half * 512 : (half + 1) * 512],
                    ot[:, 0:512],
                )

    nc.compile()
    return nc


def _get_program():
    global _PROGRAM
    if _PROGRAM is None:
        _PROGRAM = _build_program()
    return _PROGRAM


def kernel(x, kernel, bias, product_table):
    global LAST_RESULTS
    import ml_dtypes
    from concourse import bass_utils

    bf = ml_dtypes.bfloat16
    x = np.asarray(x, dtype=np.float32)
    kernel = np.asarray(kernel, dtype=np.float32)
    bias = np.asarray(bias, dtype=np.float32)
    product_table = np.asarray(product_table, dtype=np.int32)

    nc = _get_program()

    # host-tiled X^T: xt[hc, m, p, fi, j] = x[m*128+j, fi, hc*128+p]
    xt = np.ascontiguousarray(
        x.astype(bf).reshape(B // 128, 128, F_IN, 2, 128).transpose(3, 0, 4, 2, 1)
    )
    # kernel table KT[k][fi][fo]
    kt = np.ascontiguousarray(
        kernel.astype(bf).transpose(2, 1, 0)
    ).reshape(H, BLK)
    bias_row = np.ascontiguousarray(np.tile(bias, G_CORE)[None, :])

    in_maps = []
    for c in range(N_CORES):
        in_maps.append(
            {
                "xt": xt,
                "kt": kt,
                "ptg": np.ascontiguousarray(
                    product_table[:, c * G_CORE : (c + 1) * G_CORE]
                ),
                "biasrow": bias_row,
            }
        )

    res = bass_utils.run_bass_kernel_spmd(
        nc,
        in_maps,
        core_ids=list(range(N_CORES)),
        trace=TRACE,
        trace_cores=[0] if TRACE else None,
        tmpdir=os.environ.get("KERNEL_TMPDIR") or None,
    )
    LAST_RESULTS = res

    # per-core cols are (g_local, fo); assemble to (B, F_OUT, G)
    parts = [
        res.results[c]["out"].reshape(B, G_CORE, F_OUT).transpose(0, 2, 1)
        for c in range(N_CORES)
    ]
    return np.ascontiguousarray(np.concatenate(parts, axis=2), dtype=np.float32)
